# revision 55
# baseline (speedup 1.0000x reference)
"""DeeperGCN (GENConv softmax-aggr, L=2) Trainium2 kernel, 8-core SPMD.

Strategy:
  - Host permutes node ids (LPT greedy) so per-(core,block) in-degree is
    balanced, minimizing the padded per-block edge cap (CAP=CAPL+CAPH).
  - Nodes 1D-partitioned: core k owns 6250 nodes (padded to 6272 = 49*128).
  - Per layer, each core builds only m = relu(h)+eps (fp16, 128 wide),
    AllGathers the [50176, 128] table, then per dst block runs two
    dma_gathers of m rows (lo table rows [0,32768) via int16 idx, hi from
    32768), rebuilds [E|mE] = [exp(m) | m*exp(m)] post-gather on ACT/DVE,
    and scatter-accumulates via host-precomputed one-hot fp16 matmuls in
    PSUM. softmax aggregate = wsum/ssum via exp(ln(wsum)-ln(ssum)).
  - BatchNorm raw sums (sum, sum-of-squares of h1 = X@w1) accumulate
    inline during the edge phase (per-block matmul + DVE reduce + ACT
    Square), so only a [128,4] AllReduce separates edge phase from the
    single MLP pass (fp16 streams, fp32 PSUM). LayerNorm per node after
    PE transpose back to node-major.
  - Edge-phase wall time is GpSimd descriptor generation (~7.5ns/row,
    byte-size independent); everything else hides under it.
"""

import os
import sys
import math

import numpy as np

sys.path.insert(0, "/opt/trn_rl_repo")

# Problem constants (hardcoded per contract)
N = 50000
E_EDGES = 640000
D = 128
D2 = 256
L = 2
C_IN = 128
C_OUT = 64
MSG_EPS = 1e-7
W = 8           # cores
P = 128         # partitions
SH_REAL = N // W          # 6250 real nodes per core
NB = math.ceil(SH_REAL / P)   # 49 node blocks per core
SH = NB * P               # 6272 padded nodes per core
NPAD = SH * W             # 50176
BT = 250                  # BatchNorm stats tile width (SH_REAL % BT == 0)
MT = 512                  # MLP node-tile width


def default_params():
    return dict(
        W=W, P=P, D=D, D2=D2, L=L, C_OUT=C_OUT, SH=SH, SH_REAL=SH_REAL,
        NB=NB, NPAD=NPAD, BT=BT, MT=MT, MSG_EPS=MSG_EPS,
        CAPL=10, CAPH=6, LOSPLIT=32768, GRP=2, REGCNT=False,
        # fast-path flags (host-verified against actual input values)
        t_one=True, in_b_zero=True, out_b_zero=True, ln_identity=True,
        b2_zero=True,
    )


def build_program(p):
    from concourse import bacc, bass, mybir, tile
    from concourse.bass import IndirectOffsetOnAxis
    from concourse.bass_primitives import RegisterHandles
    from concourse.expressions import make_scalar_value
    from concourse.masks import make_identity
    from contextlib import ExitStack

    dt = mybir.dt
    f32, f16, i32 = dt.float32, dt.float16, dt.int32
    AF = mybir.ActivationFunctionType
    OP = mybir.AluOpType

    Wn, Pn, Dn, D2n = p["W"], p["P"], p["D"], p["D2"]
    Ln, COUT = p["L"], p["C_OUT"]
    SHn, SHR, NBn, NPADn = p["SH"], p["SH_REAL"], p["NB"], p["NPAD"]
    BTn, MTn = p["BT"], p["MT"]
    CAPL, CAPH, LOSPLIT, GRP = p["CAPL"], p["CAPH"], p["LOSPLIT"], p["GRP"]
    CAP = CAPL + CAPH
    NG = math.ceil(NBn / GRP)     # gather groups
    NBT = SHR // BTn              # bn stats tiles
    NMT = math.ceil(SHn / MTn)    # mlp node tiles
    eps_msg = p["MSG_EPS"]

    nc = bacc.Bacc(
        "TRN2", target_bir_lowering=False, debug=False,
        enable_asserts=False, num_devices=Wn, num_swdge_queues=4,
    )

    def din(name, shape, dty):
        return nc.dram_tensor(name, shape, dty, kind="ExternalInput").ap()

    i16 = dt.int16
    x_fm_d = din("x_fm", [Dn, SHn], f16)            # host-transposed x shard
    idx16_d = din("idx16", [NBn, Pn, CAP * 8], i16)  # per-block gather indices
    s_oh_d = din("s_oh", [NBn, Pn, CAP, Dn], f16)    # dst one-hot mats
    cnts_d = din("cnts", [NBn * 2], i32)             # per-block (lo16, hi16)
    in_w_d = din("in_w", [Dn, Dn], f16)
    w1_d = din("w1", [Ln, Dn, D2n], f16)
    w2_d = din("w2", [Ln, D2n, Dn], f16)
    bn_g_d = din("bn_g", [Ln, D2n], f32)
    bn_b_d = din("bn_b", [Ln, D2n], f32)
    out_w_d = din("out_w", [Dn, COUT], f16)
    if not p["b2_zero"]:
        b2_d = din("b2", [Ln, Dn], f32)
    if not p["t_one"]:
        t_d = din("t", [Ln], f32)
    if not p["in_b_zero"]:
        in_b_d = din("in_b", [Dn], f32)
    if not p["out_b_zero"]:
        out_b_d = din("out_b", [COUT], f32)
    if not p["ln_identity"]:
        ln_g_d = din("ln_g", [Ln, Dn], f32)
        ln_b_d = din("ln_b", [Ln, Dn], f32)

    out_d = nc.dram_tensor("out", [SHn, COUT], f32, kind="ExternalOutput").ap()

    rg = [list(range(Wn))]

    with ExitStack() as ctx:
        tc = ctx.enter_context(tile.TileContext(nc))
        sb = ctx.enter_context(tc.tile_pool(name="sb", bufs=1))
        sb2 = ctx.enter_context(tc.tile_pool(name="sb2", bufs=2))
        pp = ctx.enter_context(tc.tile_pool(name="pp", bufs=2, space="PSUM"))
        dr = ctx.enter_context(tc.tile_pool(name="dr", bufs=2, space="DRAM"))

        # ---- constants / weights resident in SBUF ----
        ident = sb.tile([Pn, Pn], f32, tag="ident")
        make_identity(nc, ident[:])

        in_w_sb = sb.tile([Pn, Dn], f16, tag="in_w")
        nc.sync.dma_start(out=in_w_sb[:], in_=in_w_d)
        w1_sb = sb.tile([Pn, Ln, D2n], f16, tag="w1")
        w2_sb = sb.tile([Pn, Ln, 2, Dn], f16, tag="w2")
        bng_sb = sb.tile([Pn, Ln, 2], f32, tag="bng")
        bnb_sb = sb.tile([Pn, Ln, 2], f32, tag="bnb")
        for l in range(Ln):
            nc.sync.dma_start(out=w1_sb[:, l, :], in_=w1_d[l])
            for ch in range(2):
                nc.sync.dma_start(out=w2_sb[:, l, ch, :],
                                  in_=w2_d[l, ch * Pn:(ch + 1) * Pn, :])
            nc.sync.dma_start(
                out=bng_sb[:, l, :],
                in_=bn_g_d[l].rearrange("(c p) -> p c", p=Pn))
            nc.sync.dma_start(
                out=bnb_sb[:, l, :],
                in_=bn_b_d[l].rearrange("(c p) -> p c", p=Pn))
        out_w_sb = sb.tile([Pn, COUT], f16, tag="out_w")
        nc.sync.dma_start(out=out_w_sb[:], in_=out_w_d)

        ones_row = sb.tile([1, Pn], f32, tag="ones_row")
        nc.vector.memset(ones_row[:], 1.0)

        def const_col(val, tagname):
            tcol = sb.tile([Pn, 1], f32, tag=tagname)
            nc.vector.memset(tcol[:], val)
            return tcol

        c_1e16 = const_col(1e-16, "c_1e16")
        c_1e30 = const_col(1e-30, "c_1e30")
        c_1e5 = const_col(1e-5, "c_1e5")

        def bcast_row(dram_row_ap, width, tagname):
            """[1,width] dram -> [128,width] sbuf via ones-matmul."""
            row = sb.tile([1, width], f32, tag=tagname + "_r")
            nc.sync.dma_start(out=row[:], in_=dram_row_ap)
            ps = pp.tile([Pn, width], f32, tag="psm", name=tagname + "_ps")
            nc.tensor.matmul(ps[:], lhsT=ones_row[:], rhs=row[:],
                             start=True, stop=True)
            out = sb.tile([Pn, width], f32, tag=tagname)
            nc.scalar.activation(out[:], ps[:], AF.Copy)
            return out

        if not p["b2_zero"]:
            b2c_sb = sb.tile([Pn, Ln], f32, tag="b2c")
            for l in range(Ln):
                nc.sync.dma_start(out=b2c_sb[:, l:l + 1], in_=b2_d[l][:, None])
        if not p["t_one"]:
            t_bc = bcast_row(t_d[None, :], Ln, "t_bc")  # [128, L]
        if not p["in_b_zero"]:
            inb_bc = bcast_row(in_b_d[None, :], Dn, "inb_bc")
        if not p["out_b_zero"]:
            outb_bc = bcast_row(out_b_d[None, :], COUT, "outb_bc")
        if not p["ln_identity"]:
            lng_bc = [bcast_row(ln_g_d[l][None, :], Dn, f"lng{l}")
                      for l in range(Ln)]
            lnb_bc = [bcast_row(ln_b_d[l][None, :], Dn, f"lnb{l}")
                      for l in range(Ln)]

        # ---- persistent state ----
        h_sb = sb.tile([Pn, NBn, Dn], f32, tag="h")      # node-major h shard
        X_fm = sb.tile([Pn, SHn], f16, tag="Xfm")        # feature-major agg+h
        h2T_sb = sb.tile([Pn, NBn, Dn], f32, tag="h2T")  # node-major h2

        # per-block gather counts (lo16, hi16) for runtime num_idxs
        cnt_sb = sb.tile([1, NBn * 2], i32, tag="cnts")
        nc.sync.dma_start(out=cnt_sb[:], in_=cnts_d[None, :])
        # prime gpsimd ordering after the cnts DMA (reg_load is untracked)
        prime_sb = sb.tile([1, 2], i32, tag="prime")
        nc.gpsimd.tensor_scalar(out=prime_sb[:], in0=cnt_sb[0:1, 0:2],
                                scalar1=0, scalar2=None, op0=OP.add)
        reg_lo = nc.gpsimd.alloc_register("gcnt_lo")
        reg_hi = nc.gpsimd.alloc_register("gcnt_hi")

        def load_cnt(reg, ap, mx):
            nc.gpsimd.reg_load(reg, ap)
            return make_scalar_value(RegisterHandles(reg),
                                     min_val=1, max_val=mx)
        # zero both gather-buffer instances once (reg-trimmed gathers leave
        # trailing slots unwritten; 0 -> exp=1/w=0 and S cols are 0 there)
        for _ in range(2):
            gwm0 = sb2.tile([Pn, CAP, Dn], f16, tag="gwm", name="gwm_init")
            nc.vector.memset(gwm0[:], 0.0)

        m_tabA = sb.tile([Pn, 32, Dn], f16, tag="ewA", name="m_tabA")
        m_tabB = sb.tile([Pn, NBn - 32, Dn], f16, tag="ewB", name="m_tabB")
        out_sb = sb.tile([Pn, NBn, COUT], f32, tag="out_sb")

        def emit_mtab(b):
            dst = m_tabA[:, b, :] if b < 32 else m_tabB[:, b - 32, :]
            nc.vector.tensor_scalar(
                out=dst, in0=h_sb[:, b, :], scalar1=0.0,
                scalar2=eps_msg, op0=OP.max, op1=OP.add)

        def emit_outproj(b):
            hT_ps = pp.tile([Pn, Dn], f32, tag="psm", name="hT_ps")
            nc.tensor.transpose(hT_ps[:], h_sb[:, b, :], ident[:])
            hT = sb2.tile([Pn, Dn], f16, tag="hT", name="hT")
            nc.scalar.activation(hT[:], hT_ps[:], AF.Copy)
            o_ps = pp.tile([Pn, COUT], f32, tag="psm", name="o_ps")
            nc.tensor.matmul(o_ps[:], lhsT=hT[:], rhs=out_w_sb[:],
                             start=True, stop=True)
            nc.scalar.activation(out_sb[:, b, :], o_ps[:], AF.Copy)
            if not p["out_b_zero"]:
                nc.vector.tensor_add(out_sb[:, b, :], out_sb[:, b, :],
                                     outb_bc[:])

        # ---- in-projection: h0 = x @ in_w (+ in_b) ----
        # X_fm doubles as the staging buffer for the transposed x shard;
        # the layer-0 edge phase overwrites it only after in-proj reads it.
        for c0 in range(0, SHn, SHn // 4):
            c1 = min(c0 + SHn // 4, SHn)
            nc.sync.dma_start(out=X_fm[:, c0:c1], in_=x_fm_d[:, c0:c1])
        for b in range(NBn):
            h0_ps = pp.tile([Pn, Dn], f32, tag="psm", name="h0_ps")
            nc.tensor.matmul(h0_ps[:], lhsT=X_fm[:, b * Pn:(b + 1) * Pn],
                             rhs=in_w_sb[:], start=True, stop=True)
            nc.scalar.activation(h_sb[:, b, :], h0_ps[:], AF.Copy)
            if not p["in_b_zero"]:
                nc.vector.tensor_add(h_sb[:, b, :], h_sb[:, b, :], inb_bc[:])
            emit_mtab(b)

        # ---- layers ----
        for l in range(Ln):
            bn_sum = sb.tile([Pn, 2, NBn], f32, tag="bn_sum",
                             name="bn_sum")
            bn_sq = sb.tile([Pn, 2, NBn], f32, tag="bn_sq", name="bn_sq")

            # two AllGathers aligned to the lo/hi gather tables: the lo
            # chunk (blocks 0-31) fires mid-pass2, leaving only the small
            # hi chunk's transfer exposed at the layer boundary
            mLO_full = dr.tile([Wn * 32 * Pn, Dn], f16, tag="mlo_full",
                               addr_space="Shared", name="mLO_full")
            mHI_full = dr.tile([Wn * (NBn - 32) * Pn, Dn], f16,
                               tag="mhi_full", addr_space="Shared",
                               name="mHI_full")
            for ci, (mt, mfull) in enumerate(
                    ((m_tabA, mLO_full), (m_tabB, mHI_full))):
                Lc = mt.shape[1] * Pn
                msh = dr.tile([Lc, Dn], f16, tag=f"msh{ci}",
                              name=f"msh{ci}")
                nc.sync.dma_start(
                    out=msh[:].rearrange("(b p) f -> p b f", p=Pn),
                    in_=mt[:])
                nc.gpsimd.collective_compute(
                    "AllGather", OP.bypass, replica_groups=rg,
                    ins=[msh[:]], outs=[mfull[:]])

            # -- edge aggregation: per dst block, two reg-trimmed dma_gathers
            #    of m rows (lo table rows [0,LOSPLIT), hi from LOSPLIT), then
            #    exp/mult to rebuild [E|mE], one-hot matmul accumulation --
            for b in range(NBn):
                idxt = sb2.tile([Pn, CAP * 8], i16, tag="idxt", name="idxt")
                nc.sync.dma_start(out=idxt[:], in_=idx16_d[b])
                GWm = sb2.tile([Pn, CAP, Dn], f16, tag="gwm", name="GWm")
                if p["REGCNT"]:
                    r_lo = load_cnt(reg_lo, cnt_sb[0:1, 2 * b:2 * b + 1],
                                    CAPL * Pn)
                else:
                    r_lo = CAPL * Pn
                nc.gpsimd.dma_gather(
                    out_ap=GWm[:, 0:CAPL, :], in_ap=mLO_full[:],
                    idxs_ap=idxt[:, 0:CAPL * 8],
                    num_idxs=CAPL * Pn, num_idxs_reg=r_lo, elem_size=Dn,
                    single_packet=False)
                if CAPH > 0:
                    if p["REGCNT"]:
                        r_hi = load_cnt(reg_hi,
                                        cnt_sb[0:1, 2 * b + 1:2 * b + 2],
                                        CAPH * Pn)
                    else:
                        r_hi = CAPH * Pn
                    nc.gpsimd.dma_gather(
                        out_ap=GWm[:, CAPL:CAP, :],
                        in_ap=mHI_full[:],
                        idxs_ap=idxt[:, CAPL * 8:CAP * 8],
                        num_idxs=CAPH * Pn, num_idxs_reg=r_hi, elem_size=Dn,
                        single_packet=False)
                S = sb2.tile([Pn, CAP, Dn], f16, tag="S", name="S")
                nc.sync.dma_start(out=S[:], in_=s_oh_d[b])
                GW2 = sb2.tile([Pn, CAP, 2 * Dn], f16, tag="gw2", name="GW2")
                if p["t_one"]:
                    nc.scalar.activation(GW2[:, :, 0:Dn], GWm[:], AF.Exp)
                else:
                    nc.scalar.activation(GW2[:, :, 0:Dn], GWm[:], AF.Exp,
                                         scale=t_bc[:, l:l + 1])
                nc.vector.tensor_mul(GW2[:, :, Dn:2 * Dn], GWm[:],
                                     GW2[:, :, 0:Dn])
                pblk = pp.tile([Pn, 2 * Dn], f32, tag="pblk", name="pblk")
                for c in range(CAP):
                    nc.tensor.matmul(pblk[:], lhsT=S[:, c, :],
                                     rhs=GW2[:, c, :],
                                     start=(c == 0), stop=(c == CAP - 1))
                # agg = wsum/(ssum+1e-16) = exp(ln(wsum) - ln(ssum))
                ln_e = sb2.tile([Pn, Dn], f32, tag="lne", name="ln_e")
                nc.scalar.activation(ln_e[:], pblk[:, 0:Dn], AF.Ln,
                                     bias=c_1e16[:])
                ln_w = sb2.tile([Pn, Dn], f32, tag="lnw", name="ln_w")
                nc.scalar.activation(ln_w[:], pblk[:, Dn:2 * Dn], AF.Ln,
                                     bias=c_1e30[:])
                dlog = sb2.tile([Pn, Dn], f32, tag="dlog", name="dlog")
                nc.vector.tensor_sub(dlog[:], ln_w[:], ln_e[:])
                Xnm = sb2.tile([Pn, Dn], f32, tag="Xnm", name="Xnm")
                nc.scalar.activation(Xnm[:], dlog[:], AF.Exp)
                nc.vector.tensor_add(Xnm[:], Xnm[:], h_sb[:, b, :])
                xT_ps = pp.tile([Pn, Dn], f32, tag="psm", name="xT_ps")
                nc.tensor.transpose(xT_ps[:], Xnm[:], ident[:])
                nc.scalar.activation(X_fm[:, b * Pn:(b + 1) * Pn],
                                     xT_ps[:], AF.Copy)
                # inline BN raw sums: h1_b = w1[ch]^T X_b; per-feature
                # sum and sum-of-squares over this block's nodes (DVE)
                for ch in range(2):
                    p1b = pp.tile([Pn, Pn], f32, tag="psm", name="p1b")
                    nc.tensor.matmul(
                        p1b[:], lhsT=w1_sb[:, l, ch * Pn:(ch + 1) * Pn],
                        rhs=X_fm[:, b * Pn:(b + 1) * Pn],
                        start=True, stop=True)
                    nc.vector.tensor_reduce(
                        out=bn_sum[:, ch, b:b + 1], in_=p1b[:],
                        axis=mybir.AxisListType.X, op=OP.add)
                    sq1 = sb2.tile([Pn, Pn], f32, tag="sq1", name="sq1")
                    nc.scalar.activation(sq1[:], p1b[:], AF.Square,
                                         accum_out=bn_sq[:, ch, b:b + 1])

            # -- BN stats: reduce inline raw sums over blocks; pad nodes
            #    are exactly 0 in fp16 X so sums cover the 50000 real nodes --
            bnar_sb = sb2.tile([Pn, 4], f32, tag="bnar", name="bnar_sb")
            for ch in range(2):
                nc.vector.tensor_reduce(
                    out=bnar_sb[:, ch:ch + 1], in_=bn_sum[:, ch, :],
                    axis=mybir.AxisListType.X, op=OP.add)
                nc.vector.tensor_reduce(
                    out=bnar_sb[:, 2 + ch:3 + ch], in_=bn_sq[:, ch, :],
                    axis=mybir.AxisListType.X, op=OP.add)
            bnar_in = dr.tile([Pn, 4], f32, tag="bnar_in", name="bnar_in")
            nc.sync.dma_start(out=bnar_in[:], in_=bnar_sb[:])
            bnar_out = dr.tile([Pn, 4], f32, tag="bnar_out",
                               addr_space="Shared", name="bnar_out")
            nc.gpsimd.collective_compute(
                "AllReduce", OP.add, replica_groups=rg,
                ins=[bnar_in[:]], outs=[bnar_out[:]])
            # pass2a: compute h1 = X@w1 for all tiles while the BN
            # AllReduce is in flight (only the relu needs the BN scale)
            h1_sb = sb.tile([Pn, 2, SHn], f16, tag="h1", name="h1_sb")
            for i in range(NMT):
                w_i = min(MTn, SHn - i * MTn)
                xs = X_fm[:, i * MTn:i * MTn + w_i]
                for ch in range(2):
                    p1 = pp.tile([Pn, MTn], f32, tag="mm1", name="p1a")
                    nc.tensor.matmul(
                        p1[:, :w_i],
                        lhsT=w1_sb[:, l, ch * Pn:(ch + 1) * Pn],
                        rhs=xs, start=True, stop=True)
                    nc.vector.tensor_copy(
                        h1_sb[:, ch, i * MTn:i * MTn + w_i], p1[:, :w_i])
            gsb = sb2.tile([Pn, 4], f32, tag="gsb", name="gsb")
            nc.sync.dma_start(out=gsb[:], in_=bnar_out[:])
            inv_n = 1.0 / (SHR * Wn)
            mg = sb2.tile([Pn, 2], f32, tag="mg", name="mg")
            nc.vector.tensor_scalar(out=mg[:], in0=gsb[:, 0:2],
                                    scalar1=inv_n, scalar2=None,
                                    op0=OP.mult)
            ex2 = sb2.tile([Pn, 2], f32, tag="ex2", name="ex2")
            nc.vector.tensor_scalar(out=ex2[:], in0=gsb[:, 2:4],
                                    scalar1=inv_n, scalar2=None,
                                    op0=OP.mult)
            varb = sb2.tile([Pn, 2], f32, tag="varb", name="varb")
            nc.vector.tensor_mul(varb[:], mg[:], mg[:])
            nc.vector.tensor_sub(varb[:], ex2[:], varb[:])
            lv = sb2.tile([Pn, 2], f32, tag="lv", name="lv")
            nc.scalar.activation(lv[:], varb[:], AF.Ln, bias=c_1e5[:])
            rstd = sb2.tile([Pn, 2], f32, tag="rstd", name="rstd")
            nc.scalar.activation(rstd[:], lv[:], AF.Exp, scale=-0.5)
            sc_a = sb2.tile([Pn, 2], f32, tag="sc_a", name="sc_a")
            nc.vector.tensor_mul(sc_a[:], bng_sb[:, l, :], rstd[:])
            bi_a = sb2.tile([Pn, 2], f32, tag="bi_a", name="bi_a")
            nc.vector.tensor_mul(bi_a[:], mg[:], sc_a[:])
            nc.vector.tensor_sub(bi_a[:], bnb_sb[:, l, :], bi_a[:])

            # -- MLP pass 2 + LayerNorm --
            ln_sum = sb.tile([Pn, NBn], f32, tag="ln_sum", name="ln_sum")
            ln_sq = sb.tile([Pn, NBn], f32, tag="ln_sq", name="ln_sq")

            def ln_apply(st0, st1):
                """LN batch stats + residual update + next-layer feed for
                blocks [st0, st1) — called once mid-pass2 and once at the
                end so the first half overlaps the remaining tiles."""
                wd = st1 - st0
                mu_t = sb2.tile([Pn, wd], f32, tag="mu_t", name="mu_t")
                nc.vector.tensor_scalar(out=mu_t[:], in0=ln_sum[:, st0:st1],
                                        scalar1=1.0 / Dn, scalar2=None,
                                        op0=OP.mult)
                ex2t = sb2.tile([Pn, wd], f32, tag="ex2t", name="ex2t")
                nc.vector.tensor_scalar(out=ex2t[:], in0=ln_sq[:, st0:st1],
                                        scalar1=1.0 / Dn, scalar2=None,
                                        op0=OP.mult)
                vart = sb2.tile([Pn, wd], f32, tag="vart", name="vart")
                nc.vector.tensor_mul(vart[:], mu_t[:], mu_t[:])
                nc.vector.tensor_sub(vart[:], ex2t[:], vart[:])
                lvt = sb2.tile([Pn, wd], f32, tag="lvt", name="lvt")
                nc.scalar.activation(lvt[:], vart[:], AF.Ln, bias=c_1e5[:])
                rstdt = sb2.tile([Pn, wd], f32, tag="rstdt", name="rstdt")
                nc.scalar.activation(rstdt[:], lvt[:], AF.Exp, scale=-0.5)
                Bt = sb2.tile([Pn, wd], f32, tag="Bt", name="Bt")
                nc.vector.tensor_scalar(out=Bt[:], in0=mu_t[:], scalar1=-1.0,
                                        scalar2=None, op0=OP.mult)
                nc.vector.tensor_mul(Bt[:], Bt[:], rstdt[:])
                for k in range(wd):
                    st = st0 + k
                    u = sb2.tile([Pn, Dn], f32, tag="u", name="u")
                    if p["ln_identity"]:
                        nc.scalar.activation(u[:], h2T_sb[:, st, :], AF.Relu,
                                             scale=rstdt[:, k:k + 1],
                                             bias=Bt[:, k:k + 1])
                    else:
                        nc.scalar.activation(u[:], h2T_sb[:, st, :],
                                             AF.Identity,
                                             scale=rstdt[:, k:k + 1],
                                             bias=Bt[:, k:k + 1])
                        nc.vector.tensor_mul(u[:], u[:], lng_bc[l][:])
                        nc.vector.tensor_add(u[:], u[:], lnb_bc[l][:])
                        nc.vector.tensor_scalar(out=u[:], in0=u[:],
                                                scalar1=0.0, scalar2=None,
                                                op0=OP.max)
                    nc.vector.tensor_add(h_sb[:, st, :], u[:],
                                         h_sb[:, st, :])
                    if l + 1 < Ln:
                        emit_mtab(st)
                    else:
                        emit_outproj(st)

            for i in range(NMT):
                w_i = min(MTn, SHn - i * MTn)
                hbn = []
                for ch in range(2):
                    hb = sb2.tile([Pn, MTn], f16, tag=f"hbn{ch}",
                                  name="hb")
                    nc.scalar.activation(
                        hb[:, :w_i],
                        h1_sb[:, ch, i * MTn:i * MTn + w_i], AF.Relu,
                        scale=sc_a[:, ch:ch + 1],
                        bias=bi_a[:, ch:ch + 1])
                    hbn.append(hb)
                p2 = pp.tile([Pn, MTn], f32, tag="mm2", name="p2")
                for ch in range(2):
                    nc.tensor.matmul(p2[:, :w_i], lhsT=w2_sb[:, l, ch, :],
                                     rhs=hbn[ch][:, :w_i],
                                     start=(ch == 0), stop=(ch == 1))
                h2c = sb2.tile([Pn, MTn], f32, tag="h2c", name="h2c")
                if p["b2_zero"]:
                    nc.vector.tensor_copy(h2c[:, :w_i], p2[:, :w_i])
                else:
                    nc.scalar.activation(h2c[:, :w_i], p2[:, :w_i],
                                         AF.Identity,
                                         bias=b2c_sb[:, l:l + 1])
                for j in range(w_i // Pn):
                    st = (i * MTn) // Pn + j
                    h2T_ps = pp.tile([Pn, Dn], f32, tag="psm",
                                     name="h2T_ps")
                    nc.tensor.transpose(h2T_ps[:],
                                        h2c[:, j * Pn:(j + 1) * Pn],
                                        ident[:])
                    nc.vector.tensor_copy(h2T_sb[:, st, :], h2T_ps[:])
                    nc.vector.tensor_reduce(
                        out=ln_sum[:, st:st + 1], in_=h2T_ps[:],
                        axis=mybir.AxisListType.X, op=OP.add)
                    scrap = sb2.tile([Pn, Dn], f32, tag="scrap", name="scrap")
                    nc.scalar.activation(scrap[:], h2T_ps[:], AF.Square,
                                         accum_out=ln_sq[:, st:st + 1])
                if i == 7:
                    ln_apply(0, 32)
            ln_apply(32, NBn)

        # ---- out DMA (out-projection emitted inside the last LN loop) ----
        nc.sync.dma_start(
            out=out_d.rearrange("(b p) f -> p b f", p=Pn),
            in_=out_sb[:])

    _pin_act_tables()
    _fix_swdge_bump_queues(nc)
    nc.compile()
    return nc


def _fix_swdge_bump_queues(nc):
    """Tile emits the DMASW sem-bump (InstIncSwdgeSem) for prepare_only
    SWDGE preps with queue_num=0 regardless of the prep's queue. Our preps
    cycle queues exactly like Tile cycles DMASW lanes (j % 4), so lane i's
    bump belongs on queue i."""
    from concourse import bass_isa
    for b in nc.main_func.blocks:
        for i in b.instructions:
            if isinstance(i, bass_isa.InstIncSwdgeSem) and i._mode == "add":
                names = i._sem_names
                if names and names[0].startswith("DMASW"):
                    lane = int(names[0][5:].split("_")[0])
                    i.queue_num = lane % 4


def _pin_act_tables():
    """Force all activation funcs onto natural_log_exp_and_others so the
    kernel needs exactly one ACT table load (Exp/Ln/Copy/Relu/Identity are
    all members). Default placement ping-pongs exp_and_others <->
    natural_log, costing ~1.3us per switch."""
    import concourse.bacc as bacc_mod
    import concourse.hw_specs as hw_specs_mod
    if getattr(bacc_mod, "_act_tables_pinned", False):
        return
    orig = hw_specs_mod.get_activation_tables

    def pinned(arch):
        t = orig(arch)
        keep = "natural_log_exp_and_others"
        return {name: (fns if name == keep else set())
                for name, fns in t.items()}

    bacc_mod.get_activation_tables = pinned
    bacc_mod._act_tables_pinned = True


# ---------------------------------------------------------------------------
# Host-side data prep
# ---------------------------------------------------------------------------

def balance_perm(edge_index, p):
    """Permute node ids so per-(core,block) in-degree is balanced (LPT
    greedy into 392 bins), lowering the padded gather/matmul cap.
    Returns perm (old id -> new id)."""
    import heapq
    Wn, Pn, NBn, SHR = p["W"], p["P"], p["NB"], p["SH_REAL"]
    n = Wn * SHR
    deg = np.bincount(np.asarray(edge_index[1]), minlength=n)
    order = np.argsort(-deg, kind="stable")
    nbins = Wn * NBn
    cap = np.full(nbins, Pn, np.int64)
    cap[NBn - 1::NBn] = SHR - (NBn - 1) * Pn      # last block per core
    heap = [(0, b) for b in range(nbins)]
    heapq.heapify(heap)
    fill = np.zeros(nbins, np.int64)
    perm = np.empty(n, np.int64)
    dsorted = deg[order]
    for i in range(n):
        s, b = heapq.heappop(heap)
        c = fill[b]
        fill[b] = c + 1
        core, blk = divmod(b, NBn)
        perm[order[i]] = core * SHR + blk * Pn + c
        if c + 1 < cap[b]:
            heapq.heappush(heap, (s + int(dsorted[i]), b))
    return perm


def prep_edges(edge_index, p):
    """Group edges by (dst core, dst block), split each block's edges into a
    lo segment (src row < LOSPLIT) and a hi segment, pad each segment's count
    up to a multiple of 16 (pad gather idx 0, pad one-hot col 200), and build
    per-block wrapped int16 gather indices, one-hot scatter matrices, and
    (lo16, hi16) runtime gather counts."""
    Wn, Pn, NBn = p["W"], p["P"], p["NB"]
    SHR, SHn, LOSPLIT = p["SH_REAL"], p["SH"], p["LOSPLIT"]
    src = edge_index[0].astype(np.int64)
    dst = edge_index[1].astype(np.int64)
    sk = src // SHR
    sj = src % SHR
    # chunked table layout: chunk c of each shard (blocks 0-15/16-31/32-48)
    # is AllGathered to base 8*b0*128 with per-core stride = chunk length
    src_pad = np.where(sj < 4096, sk * 4096 + sj,
                       32768 + sk * 2176 + (sj - 4096))
    core = dst // SHR
    dstl = dst % SHR
    blk = dstl // Pn
    col = (dstl % Pn).astype(np.float32)
    hi = (src_pad >= LOSPLIT).astype(np.int64)
    # order edges by (core, block, hi) so each segment is contiguous
    key = (core * NBn + blk) * 2 + hi
    order = np.lexsort((src_pad, key))
    counts = np.bincount(key, minlength=Wn * NBn * 2)
    cl = counts[0::2].reshape(Wn, NBn)
    ch = counts[1::2].reshape(Wn, NBn)
    CAPL = max(1, int(math.ceil(cl.max() / Pn)))
    CAPH = int(math.ceil(ch.max() / Pn))
    CAP = CAPL + CAPH
    starts = np.zeros(Wn * NBn * 2, np.int64)
    starts[1:] = np.cumsum(counts)[:-1]
    ne = len(src)
    ko = key[order]
    pos = np.arange(ne) - starts[ko]          # position within segment
    # slot index within the (core, block) padded layout:
    #  lo edges:   slot = pos           (< CAPL*128)
    #  hi edges:   slot = CAPL*128 + pos
    slot = pos + (ko % 2) * CAPL * Pn
    cb = ko // 2                               # core*NB + blk
    # gather index value: row within its table (lo: src_pad, hi: -LOSPLIT)
    gidx = (src_pad[order] - hi[order] * LOSPLIT).astype(np.int16)
    idxs = np.zeros((Wn * NBn, CAP * Pn), np.int16)
    colb = np.full((Wn * NBn, CAP * Pn), 200.0, np.float32)
    idxs[cb, slot] = gidx
    colb[cb, slot] = col[order]
    cnts = np.stack([np.maximum(cl, 1), np.maximum(ch, 1)],
                    axis=2).reshape(Wn, NBn * 2).astype(np.int32)
    cnts = np.ascontiguousarray(cnts)
    # one-hot scatter matrices: [W, NB, 128, CAP, 128] fp16
    colt = colb.reshape(Wn, NBn, CAP, Pn).transpose(0, 1, 3, 2)
    s_oh = np.ascontiguousarray(
        (colt[..., None] == np.arange(Pn, dtype=np.float32)
         ).astype(np.float16))
    # per-block wrapped gather indices: [W, NB, 128, CAP*8]
    nflat = CAP * Pn
    wrapped = np.zeros((Wn * NBn, 16, nflat // 16), np.int16)
    ii = np.arange(nflat)
    wrapped[:, ii % 16, ii // 16] = idxs
    idx16 = np.ascontiguousarray(
        np.tile(wrapped, (1, 8, 1)).reshape(Wn, NBn, Pn, CAP * 8))
    return idx16, s_oh, cnts, CAPL, CAPH


def prep_in_maps(inputs, p, idx16, s_oh, cnts):
    Wn, Pn = p["W"], p["P"]
    SHR, SHn = p["SH_REAL"], p["SH"]
    x = np.asarray(inputs["x"], np.float32)
    in_maps = []
    for k in range(Wn):
        xs = np.zeros((SHn, x.shape[1]), np.float32)
        xs[:SHR] = x[k * SHR:(k + 1) * SHR]
        m = {
            "x_fm": np.ascontiguousarray(xs.T.astype(np.float16)),
            "idx16": idx16[k],
            "s_oh": s_oh[k],
            "cnts": cnts[k],
            "in_w": np.asarray(inputs["in_w"], np.float16),
            "w1": np.asarray(inputs["w1"], np.float16),
            "w2": np.asarray(inputs["w2"], np.float16),
            "bn_g": np.asarray(inputs["bn_g"], np.float32),
            "bn_b": np.asarray(inputs["bn_b"], np.float32),
            "out_w": np.asarray(inputs["out_w"], np.float16),
        }
        if not p["b2_zero"]:
            m["b2"] = np.asarray(inputs["b2"], np.float32)
        if not p["t_one"]:
            m["t"] = np.asarray(inputs["t"], np.float32)
        if not p["in_b_zero"]:
            m["in_b"] = np.asarray(inputs["in_b"], np.float32)
        if not p["out_b_zero"]:
            m["out_b"] = np.asarray(inputs["out_b"], np.float32)
        if not p["ln_identity"]:
            m["ln_g"] = np.asarray(inputs["ln_g"], np.float32)
            m["ln_b"] = np.asarray(inputs["ln_b"], np.float32)
        in_maps.append(m)
    return in_maps


def detect_fastpath(inputs, p):
    p["t_one"] = bool(np.all(np.asarray(inputs["t"]) == 1.0))
    p["in_b_zero"] = bool(np.all(np.asarray(inputs["in_b"]) == 0.0))
    p["out_b_zero"] = bool(np.all(np.asarray(inputs["out_b"]) == 0.0))
    p["b2_zero"] = bool(np.all(np.asarray(inputs["b2"]) == 0.0))
    p["ln_identity"] = bool(
        np.all(np.asarray(inputs["ln_g"]) == 1.0)
        and np.all(np.asarray(inputs["ln_b"]) == 0.0))
    # b1 is skipped unconditionally: it cancels exactly in BatchNorm.
    return p


_PROGRAM_CACHE = {}


def _get_program(p):
    key = (p["CAPL"], p["CAPH"], p["t_one"], p["in_b_zero"],
           p["out_b_zero"], p["b2_zero"], p["ln_identity"], p["REGCNT"])
    if key not in _PROGRAM_CACHE:
        _PROGRAM_CACHE[key] = build_program(p)
    return _PROGRAM_CACHE[key]


def _ensure_ntff_hook():
    """Register the axon NTFF profiling hook (the image's antenv package
    lacks axon_hooks; inject an equivalent module)."""
    import types
    if "antenv.axon_hooks" in sys.modules:
        return
    sys.path.insert(0, "/root/.axon_site")
    from trn_agent_boot.trn_boot import _ntff_profile_via_ctypes
    hook = _ntff_profile_via_ctypes("/opt/axon/libaxon_pjrt.so")
    mod = types.ModuleType("antenv.axon_hooks")
    mod._hook = hook
    mod.set_axon_ntff_profile_hook = lambda h: setattr(mod, "_hook", h)
    mod.get_axon_ntff_profile_hook = lambda: mod._hook
    sys.modules["antenv.axon_hooks"] = mod


def run(inputs, trace=False, trace_cores=None):
    from concourse.bass_utils import run_bass_kernel_spmd
    if trace:
        _ensure_ntff_hook()
    p = default_params()
    detect_fastpath(inputs, p)
    ei = np.asarray(inputs["edge_index"])
    perm = balance_perm(ei, p)
    ei_p = perm[ei.astype(np.int64)].astype(np.int32)
    x = np.asarray(inputs["x"], np.float32)
    x_p = np.empty_like(x)
    x_p[perm] = x
    inputs = dict(inputs)
    inputs["x"] = x_p
    idx16, s_oh, cnts, CAPL, CAPH = prep_edges(ei_p, p)
    p["CAPL"], p["CAPH"] = CAPL, CAPH
    nc = _get_program(p)
    in_maps = prep_in_maps(inputs, p, idx16, s_oh, cnts)
    kwargs = {}
    if trace:
        kwargs = dict(trace=True,
                      trace_cores=trace_cores or [0])
    bkr = run_bass_kernel_spmd(nc, in_maps, core_ids=list(range(p["W"])),
                               **kwargs)
    outs = []
    for k in range(p["W"]):
        outs.append(np.asarray(bkr.results[k]["out"])[:p["SH_REAL"]])
    full = np.concatenate(outs, axis=0).astype(np.float32)[perm]
    return full, bkr


def kernel(**inputs):
    full, _ = run(inputs, trace=False)
    return full

